# revision 3
# baseline (speedup 1.0000x reference)
"""Trainium2 Bass kernel for DPPDynamicEmbedding (retrieval_knn).

Reference computation (per batch b, N=4096 points in [0,1]^2):
  placed    = (~action_mask) & ~(keepout | probe)                  [N] bool
  d2[i,j]   = |x_i|^2 + |x_j|^2 - 2 x_i.x_j                        [N,N]
  density_i = |{j : placed_j and d2[i,j] < R^2}| / 20              [N]
  proj      = [placed, density] @ W                                [N, 384]
  out       = split(proj, 3) -> (glimpse_key, glimpse_val, logit_key)

Strategy: data-parallel, 2 batches per core on 8 cores.  On device, per
i-block of 128 points (i = p*C + ib, p = SBUF partition, C = N/128):
  - PE computes psum[i, j] = -2 x_i.x_j + masked_sq_j  via a K=3 matmul
    (lhsT rows = [xi0, xi1, 1], rhs rows = [-2 xj0, -2 xj1, sq_j or BIG]),
    with the j-domain compacted on the host to only placed points (padded
    to J, pad entries get sq = BIG so they never count).
  - The comparison d2 < R^2  <=>  psum < thresh_i  (thresh = R^2 - sq_i)
    is a single fused compare+count per j-chunk:
      ACT: Sign(thresh - psum) with accum_out  (count = (S + width)/2)
      DVE: tensor_scalar is_lt with accum_out  (count = S)
    split across both engines for balance.
  - Counts [128, C] go through a tiny DRAM round-trip to become a
    row [1, N] (p-major flatten == natural i order), forming feat^T rows.
  - Projection is a K=(2+nchunks) matmul: lhsT = [placed; acc_0..; ones],
    rhs = [W0; W1/20 or W1/40 per chunk; (J_act/40) W1], so the count
    normalization and the Sign->count affine fix ride the matmul for free.
  - PSUM result is copied to SBUF (alternating ACT/DVE) and DMA'd out.
"""

import numpy as np

import concourse.bass as bass
import concourse.mybir as mybir
import concourse.tile as tile
from concourse import bacc, bass_utils

R2 = 0.16
SCALE = 20.0
BIG = 1.0e9
N_CORES = 8

F32 = mybir.dt.float32
BF16 = mybir.dt.bfloat16


def _chunk_plan(J):
    """Split [0, J) into j-chunks of width <=512 assigned to ACT or DVE,
    greedily balancing the per-i-block cost model (ns)."""
    widths = [512] * (J // 512)
    if J % 512:
        widths.append(J % 512)
    # output-copy cost [128,384] f32 is assigned later; bias ACT slightly.
    t_act = 0.0
    t_dve = 120.0  # DVE also does misc small work
    chunks = []
    for w in widths:
        ca = (172 + w) / 1.2
        cd = (120 + w) / 0.96
        if t_act + ca <= t_dve + cd:
            chunks.append((w, "act"))
            t_act += ca
        else:
            chunks.append((w, "dve"))
            t_dve += cd
    return chunks


def build_program(N, BPC, J, chunks, copy_split=9):
    """Build the Bass program.  N points, BPC batches per core, J padded
    placed count, chunks = [(width, 'act'|'dve'), ...]."""
    C = N // 128          # columns per partition in p-major layout
    NB = N // 128         # number of 128-point i-blocks per batch
    NG = len(chunks)
    K_PROJ = 2 + NG

    nc = bacc.Bacc("TRN2", target_bir_lowering=False, debug=False,
                   num_devices=N_CORES)

    xi_d = nc.dram_tensor("xi", [BPC, 3, N], F32, kind="ExternalInput")
    rhs3_d = nc.dram_tensor("rhs3", [BPC, 3, J], F32, kind="ExternalInput")
    th_d = nc.dram_tensor("thresh", [BPC, 128, C], F32, kind="ExternalInput")
    ft_d = nc.dram_tensor("featT", [BPC, K_PROJ, N], F32, kind="ExternalInput")
    rw_d = nc.dram_tensor("rhsW", [K_PROJ, 384], F32, kind="ExternalInput")

    gk_d = nc.dram_tensor("gk", [BPC, N, 128], F32, kind="ExternalOutput")
    gv_d = nc.dram_tensor("gv", [BPC, N, 128], F32, kind="ExternalOutput")
    lk_d = nc.dram_tensor("lk", [BPC, N, 128], F32, kind="ExternalOutput")
    out_ds = [gk_d, gv_d, lk_d]

    with tile.TileContext(nc) as tc:
        with (
            tc.tile_pool(name="const", bufs=2 * BPC) as cpool,
            tc.tile_pool(name="accp", bufs=NG * BPC) as accp,
            tc.tile_pool(name="pd", bufs=4, space="PSUM") as pdp,
            tc.tile_pool(name="po", bufs=2, space="PSUM") as pop,
            tc.tile_pool(name="scr_a", bufs=2) as scra,
            tc.tile_pool(name="scr_d", bufs=2) as scrd,
            tc.tile_pool(name="outsb", bufs=3) as outp,
            tc.tile_pool(name="dram", bufs=NG * BPC, space="DRAM") as dram,
            tc.tile_pool(name="w", bufs=1) as wpool,
        ):
            rhsW = wpool.tile([K_PROJ, 384], F32)
            nc.sync.dma_start(rhsW[:], rw_d.ap())

            xi, rhs3, th, ft, accs, scratch_d = [], [], [], [], [], []
            for b in range(BPC):
                t = cpool.tile([3, N], F32, tag="xi")
                nc.sync.dma_start(t[:], xi_d.ap()[b])
                xi.append(t)
                t = cpool.tile([3, J], F32, tag="rhs3")
                nc.sync.dma_start(t[:], rhs3_d.ap()[b])
                rhs3.append(t)
                t = cpool.tile([128, C], F32, tag="th")
                nc.sync.dma_start(t[:], th_d.ap()[b])
                th.append(t)
                t = cpool.tile([K_PROJ, N], F32, tag="ft")
                nc.sync.dma_start(t[:], ft_d.ap()[b])
                ft.append(t)
                accs.append([accp.tile([128, C], F32, tag=f"acc{g}",
                                        name=f"acc{g}_{b}")
                             for g in range(NG)])
                scratch_d.append([dram.tile([128, C], F32, tag=f"sd{g}",
                                            name=f"sd{g}_{b}")
                                  for g in range(NG)])

            # ---- phase 1: counts ----
            for b in range(BPC):
                for ib in range(NB):
                    lhs = xi[b][:, ib * 128:(ib + 1) * 128]
                    off = 0
                    for g, (w, eng) in enumerate(chunks):
                        pd = pdp.tile([128, 512], F32, tag="pd")
                        nc.tensor.matmul(pd[:, :w], lhs,
                                         rhs3[b][:, off:off + w],
                                         start=True, stop=True)
                        acc_col = accs[b][g][:, ib:ib + 1]
                        if eng == "act":
                            scr = scra.tile([128, 512], BF16, tag="sa")
                            nc.scalar.activation(
                                scr[:, :w], pd[:, :w],
                                mybir.ActivationFunctionType.Sign,
                                bias=th[b][:, ib:ib + 1], scale=-1.0,
                                accum_out=acc_col)
                        else:
                            scr = scrd.tile([128, 512], BF16, tag="sd")
                            nc.vector.tensor_scalar(
                                scr[:, :w], pd[:, :w],
                                th[b][:, ib:ib + 1], None,
                                op0=mybir.AluOpType.is_lt,
                                op1=mybir.AluOpType.add,
                                accum_out=acc_col)
                        off += w

                # counts -> DRAM -> featT rows (p-major flatten == i order)
                for g in range(NG):
                    nc.sync.dma_start(scratch_d[b][g][:, :], accs[b][g][:])
                    row = scratch_d[b][g][:, :].rearrange("p c -> (p c)") \
                        .rearrange("(a n) -> a n", a=1)
                    nc.sync.dma_start(ft[b][1 + g:2 + g, :], row)

            # ---- phase 2: projection ----
            for b in range(BPC):
                for ib in range(NB):
                    po = pop.tile([128, 384], F32, tag="po")
                    nc.tensor.matmul(po[:], ft[b][:, ib * 128:(ib + 1) * 128],
                                     rhsW[:], start=True, stop=True)
                    osb = outp.tile([128, 384], F32, tag="osb")
                    if (ib % 16) < copy_split:
                        nc.scalar.copy(osb[:], po[:])
                    else:
                        nc.vector.tensor_copy(osb[:], po[:])
                    for o in range(3):
                        nc.sync.dma_start(
                            out_ds[o].ap()[b, ib * 128:(ib + 1) * 128, :],
                            osb[:, o * 128:(o + 1) * 128])
    nc.compile()
    return nc


def prep_core_inputs(action_mask, keepout, probe, locs, W, J, chunks):
    """Host-side prep for one core's batches. Returns in_map dict."""
    BPC, N, _ = locs.shape
    C = N // 128
    NG = len(chunks)
    K_PROJ = 2 + NG

    placed = (~action_mask) & ~(keepout | probe)          # [BPC, N] bool
    placed_f = placed.astype(np.float32)
    sq = (locs.astype(np.float32) ** 2).sum(-1)           # [BPC, N]
    thresh = (R2 - sq).astype(np.float32)                 # [BPC, N]

    # p-major i-block permutation: lhsT col m of block ib <- i = m*C + ib
    m = np.arange(128)
    perm = (np.arange(C)[:, None] * 128 + m[None, :]).reshape(-1)  # ib*128+m
    src = (m[None, :] * C + np.arange(C)[:, None]).reshape(-1)     # m*C+ib

    xi = np.zeros((BPC, 3, N), np.float32)
    rhs3 = np.full((BPC, 3, J), 0.0, np.float32)
    rhs3[:, 2, :] = BIG
    featT = np.zeros((BPC, K_PROJ, N), np.float32)
    th_pm = np.zeros((BPC, 128, C), np.float32)

    for b in range(BPC):
        x = locs[b].astype(np.float32)
        xi[b, 0, perm] = x[src, 0]
        xi[b, 1, perm] = x[src, 1]
        xi[b, 2, :] = 1.0
        idx = np.nonzero(placed[b])[0]
        assert len(idx) <= J, f"placed count {len(idx)} exceeds J={J}"
        rhs3[b, 0, :len(idx)] = -2.0 * x[idx, 0]
        rhs3[b, 1, :len(idx)] = -2.0 * x[idx, 1]
        rhs3[b, 2, :len(idx)] = sq[b, idx]
        featT[b, 0, :] = placed_f[b]
        featT[b, K_PROJ - 1, :] = 1.0
        th_pm[b] = thresh[b].reshape(128, C)

    W = W.astype(np.float32)
    rhsW = np.zeros((K_PROJ, 384), np.float32)
    rhsW[0] = W[0]
    j_act = 0
    for g, (w, eng) in enumerate(chunks):
        if eng == "act":
            rhsW[1 + g] = W[1] / (2.0 * SCALE)
            j_act += w
        else:
            rhsW[1 + g] = W[1] / SCALE
    rhsW[K_PROJ - 1] = (j_act / (2.0 * SCALE)) * W[1]

    return {"xi": xi, "rhs3": rhs3, "thresh": th_pm, "featT": featT,
            "rhsW": rhsW}


_PROGRAM_CACHE = {}


def kernel(action_mask, keepout, probe, locs, W, _trace=False, _tmpdir=None):
    action_mask = np.asarray(action_mask)
    keepout = np.asarray(keepout)
    probe = np.asarray(probe)
    locs = np.asarray(locs, dtype=np.float32)
    W = np.asarray(W, dtype=np.float32)

    B, N = action_mask.shape
    BPC = B // N_CORES

    placed = (~action_mask) & ~(keepout | probe)
    max_placed = int(placed.sum(1).max())
    J = max(1024, ((max_placed + 63) // 64) * 64)
    chunks = _chunk_plan(J)

    key = (N, BPC, J, tuple(chunks))
    if key not in _PROGRAM_CACHE:
        _PROGRAM_CACHE[key] = build_program(N, BPC, J, chunks)
    nc = _PROGRAM_CACHE[key]

    in_maps = []
    for c in range(N_CORES):
        s = slice(c * BPC, (c + 1) * BPC)
        in_maps.append(prep_core_inputs(
            action_mask[s], keepout[s], probe[s], locs[s], W, J, chunks))

    res = bass_utils.run_bass_kernel_spmd(
        nc, in_maps, core_ids=list(range(N_CORES)),
        trace=_trace, tmpdir=_tmpdir)

    gk = np.concatenate([res.results[c]["gk"] for c in range(N_CORES)], 0)
    gv = np.concatenate([res.results[c]["gv"] for c in range(N_CORES)], 0)
    lk = np.concatenate([res.results[c]["lk"] for c in range(N_CORES)], 0)
    out = (gk.astype(np.float32), gv.astype(np.float32),
           lk.astype(np.float32))
    if _trace:
        return out, res
    return out


# revision 5
# speedup vs baseline: 2.8128x; 2.8128x over previous
"""Trainium2 Bass kernel for DPPDynamicEmbedding (retrieval_knn).

Reference computation (per batch b, N=4096 points in [0,1]^2):
  placed    = (~action_mask) & ~(keepout | probe)                  [N] bool
  d2[i,j]   = |x_i|^2 + |x_j|^2 - 2 x_i.x_j                        [N,N]
  density_i = |{j : placed_j and d2[i,j] < R^2}| / 20              [N]
  proj      = [placed, density] @ W                                [N, 384]
  out       = split(proj, 3) -> (glimpse_key, glimpse_val, logit_key)

Strategy: data-parallel, 2 batches per core on 8 cores.  Per i-block of
128 points (i = p*C + ib, p = SBUF partition, C = N/128):

  - PE computes psum[i, j] = -2 x_i.x_j + masked_sq_j with a K=8 fp16
    matmul.  fp16 hi/lo splitting (x = xh + xl, sq = sqh + sql) keeps
    d2 accurate to ~1e-6 while running single-pass (fp32 matmuls are
    2-pass on TRN2 and much slower end to end).  The j-domain is
    compacted on the host to only placed points, padded to J; pad
    entries get sq = 32768 so they can never be inside the radius.
  - The comparison d2 < R^2  <=>  psum < thresh_i (thresh = R^2 - sq_i)
    is one fused compare+count op per engine per i-block, reading a
    2-bank PSUM tile:
      ACT: Sign(thresh - psum), accum_out S  => count = (S + Wa)/2
      DVE: tensor_scalar is_lt, accum_out C  => count = C
  - Counts (fp32, converted to exact fp16 integers) take a tiny DRAM
    round-trip to become rows [1, N] (p-major flatten == i order) of
    feat^T.
  - Projection is one K=8 fp16 matmul per 128 points: lhsT rows =
    [placed, placed, S, S, C, C, 1, 1], rhs rows = hi/lo splits of
    [W0, W1/40, W1/20, (Wa/40) W1] -- count normalization and the
    Sign->count affine fix ride the matmul; the hi/lo W split keeps
    fp32-level accuracy.  PSUM -> SBUF copy (ACT/DVE split; DMA cannot
    read PSUM) then one DMA per two i-blocks into a [N, 384] output.
"""

import numpy as np

import concourse.bass as bass
import concourse.mybir as mybir
import concourse.tile as tile
from concourse import bacc, bass_utils

R2 = 0.16
SCALE = 20.0
BIG = 32768.0          # pad sentinel; must be fp16-exact and >> R2
N_CORES = 8

F32 = mybir.dt.float32
F16 = mybir.dt.float16


def _split16(v):
    """Split fp32 array into (hi, lo) fp16 pair with v ~= hi + lo."""
    hi = v.astype(np.float16)
    lo = (v - hi.astype(np.float32)).astype(np.float16)
    return hi, lo


def _wa_for(J):
    """ACT-side j-width (DVE gets J - wa)."""
    if J >= 1792:
        return 1024
    return max(512, (int(J * 0.55) // 512) * 512)


def _subchunks(w):
    out = []
    off = 0
    while off < w:
        out.append((off, min(512, w - off)))
        off += 512
    return out


def build_program(N, BPC, J, wa, copy_split=6):
    """N points, BPC batches per core, J padded j-count, wa = ACT width."""
    C = N // 128
    NB = N // 128
    wd = J - wa

    nc = bacc.Bacc("TRN2", target_bir_lowering=False, debug=False,
                   num_devices=N_CORES)

    xi_d = nc.dram_tensor("xi", [BPC, 8, N], F16, kind="ExternalInput")
    rhs3_d = nc.dram_tensor("rhs3", [BPC, 8, J], F16, kind="ExternalInput")
    th_d = nc.dram_tensor("thresh", [BPC, 128, C], F32, kind="ExternalInput")
    ft_d = nc.dram_tensor("featT", [BPC, 8, N], F16, kind="ExternalInput")
    rw_d = nc.dram_tensor("rhsW", [8, 384], F16, kind="ExternalInput")
    pj_d = nc.dram_tensor("proj", [BPC, N, 384], F32, kind="ExternalOutput")

    with tile.TileContext(nc) as tc:
        with (
            tc.tile_pool(name="const", bufs=BPC) as cpool,
            tc.tile_pool(name="accp", bufs=BPC) as accp,
            tc.tile_pool(name="pa", bufs=2, space="PSUM") as pap,
            tc.tile_pool(name="pdv", bufs=1, space="PSUM") as pdp,
            tc.tile_pool(name="po", bufs=2, space="PSUM") as pop,
            tc.tile_pool(name="scr_a", bufs=2) as scra,
            tc.tile_pool(name="scr_d", bufs=2) as scrd,
            tc.tile_pool(name="outsb", bufs=2) as outp,
            tc.tile_pool(name="dram", bufs=BPC, space="DRAM") as dram,
            tc.tile_pool(name="w", bufs=1) as wpool,
        ):
            rhsW = wpool.tile([8, 384], F16)
            nc.sync.dma_start(rhsW[:], rw_d.ap())

            xi, rhs3, th, ft, acc_a, acc_d, sd_a, sd_d = \
                [], [], [], [], [], [], [], []
            for b in range(BPC):
                t = cpool.tile([8, N], F16, tag="xi", name=f"xi{b}")
                nc.sync.dma_start(t[:], xi_d.ap()[b])
                xi.append(t)
                t = cpool.tile([8, J], F16, tag="rhs3", name=f"rhs3{b}")
                nc.sync.dma_start(t[:], rhs3_d.ap()[b])
                rhs3.append(t)
                t = cpool.tile([128, C], F32, tag="th", name=f"th{b}")
                nc.sync.dma_start(t[:], th_d.ap()[b])
                th.append(t)
                t = cpool.tile([8, N], F16, tag="ft", name=f"ft{b}")
                nc.sync.dma_start(t[:], ft_d.ap()[b])
                ft.append(t)
                acc_a.append(accp.tile([128, C], F32, tag="aa", name=f"aa{b}"))
                acc_d.append(accp.tile([128, C], F32, tag="ad", name=f"ad{b}"))
                sd_a.append(dram.tile([128, C], F16, tag="sa", name=f"sda{b}"))
                sd_d.append(dram.tile([128, C], F16, tag="sd", name=f"sdd{b}"))

            # ---- phase 1: counts ----
            for b in range(BPC):
                for ib in range(NB):
                    lhs = xi[b][:, ib * 128:(ib + 1) * 128]
                    pa = pap.tile([128, 1024], F32, tag="pa")
                    for off, w in _subchunks(wa):
                        nc.tensor.matmul(pa[:, off:off + w], lhs,
                                         rhs3[b][:, off:off + w],
                                         start=True, stop=True)
                    pd = pdp.tile([128, 1024], F32, tag="pd")
                    for off, w in _subchunks(wd):
                        nc.tensor.matmul(pd[:, off:off + w], lhs,
                                         rhs3[b][:, wa + off:wa + off + w],
                                         start=True, stop=True)
                    sa = scra.tile([128, 1024], F16, tag="sa")
                    nc.scalar.activation(
                        sa[:, :wa], pa[:, :wa],
                        mybir.ActivationFunctionType.Sign,
                        bias=th[b][:, ib:ib + 1], scale=-1.0,
                        accum_out=acc_a[b][:, ib:ib + 1])
                    sd = scrd.tile([128, 1024], F16, tag="sd")
                    nc.vector.tensor_scalar(
                        sd[:, :wd], pd[:, :wd],
                        th[b][:, ib:ib + 1], None,
                        op0=mybir.AluOpType.is_lt,
                        op1=mybir.AluOpType.add,
                        accum_out=acc_d[b][:, ib:ib + 1])

                # counts: fp32 -> exact fp16 ints -> DRAM -> featT rows
                a16 = accp.tile([128, C], F16, tag="a16", name=f"a16_{b}")
                nc.vector.tensor_copy(a16[:], acc_a[b][:])
                d16 = accp.tile([128, C], F16, tag="d16", name=f"d16_{b}")
                nc.vector.tensor_copy(d16[:], acc_d[b][:])
                nc.sync.dma_start(sd_a[b][:, :], a16[:])
                nc.sync.dma_start(sd_d[b][:, :], d16[:])
                row_a = sd_a[b][:, :].rearrange("p c -> (p c)") \
                    .rearrange("(a n) -> a n", a=1)
                row_d = sd_d[b][:, :].rearrange("p c -> (p c)") \
                    .rearrange("(a n) -> a n", a=1)
                # featT rows: [pl, pl, S, S, C, C, 1, 1]
                nc.sync.dma_start(ft[b][2:3, :], row_a)
                nc.sync.dma_start(ft[b][3:4, :], row_a)
                nc.sync.dma_start(ft[b][4:5, :], row_d)
                nc.sync.dma_start(ft[b][5:6, :], row_d)

            # ---- phase 2: projection ----
            for b in range(BPC):
                for ib2 in range(0, NB, 2):
                    osb = outp.tile([128, 768], F32, tag="osb")
                    for s in range(2):
                        ib = ib2 + s
                        po = pop.tile([128, 384], F32, tag="po")
                        nc.tensor.matmul(
                            po[:], ft[b][:, ib * 128:(ib + 1) * 128],
                            rhsW[:], start=True, stop=True)
                        if (ib % 8) < copy_split:
                            nc.vector.tensor_copy(
                                osb[:, s * 384:(s + 1) * 384], po[:])
                        else:
                            nc.scalar.copy(
                                osb[:, s * 384:(s + 1) * 384], po[:])
                    dst = pj_d.ap()[b, ib2 * 128:(ib2 + 2) * 128, :] \
                        .rearrange("(s p) k -> p s k", p=128)
                    nc.sync.dma_start(
                        dst, osb[:].rearrange("p (s k) -> p s k", s=2))
    nc.compile()
    return nc


def prep_core_inputs(action_mask, keepout, probe, locs, W, J, wa):
    """Host-side prep for one core's batches. Returns in_map dict."""
    BPC, N, _ = locs.shape
    C = N // 128

    placed = (~action_mask) & ~(keepout | probe)          # [BPC, N] bool
    placed_f = placed.astype(np.float32)
    x = locs.astype(np.float32)
    sq = (x ** 2).sum(-1)                                 # [BPC, N]
    thresh = (R2 - sq).astype(np.float32)

    # p-major i-block layout: lhsT col m of block ib <- i = m*C + ib
    m = np.arange(128)
    src = (m[None, :] * C + np.arange(C)[:, None]).reshape(-1)  # pos ib*128+m

    xi = np.zeros((BPC, 8, N), np.float16)
    rhs3 = np.zeros((BPC, 8, J), np.float16)
    rhs3[:, 6, :] = BIG
    featT = np.zeros((BPC, 8, N), np.float16)
    th_pm = np.zeros((BPC, 128, C), np.float32)

    for b in range(BPC):
        x0h, x0l = _split16(x[b, :, 0])
        x1h, x1l = _split16(x[b, :, 1])
        # lhsT rows: [xh0, xh0, xl0, xh1, xh1, xl1, 1, 1]
        xi[b, 0, :] = x0h[src]
        xi[b, 1, :] = x0h[src]
        xi[b, 2, :] = x0l[src]
        xi[b, 3, :] = x1h[src]
        xi[b, 4, :] = x1h[src]
        xi[b, 5, :] = x1l[src]
        xi[b, 6, :] = 1.0
        xi[b, 7, :] = 1.0

        idx = np.nonzero(placed[b])[0]
        np_ = len(idx)
        assert np_ <= J, f"placed count {np_} exceeds J={J}"
        j0h, j0l = _split16(-2.0 * x[b, idx, 0])
        j1h, j1l = _split16(-2.0 * x[b, idx, 1])
        sqh, sql = _split16(sq[b, idx])
        # rhs rows paired with lhsT rows:
        # [-2xh0_j, -2xl0_j... wait: pairs give xh*(-2xh), xh*(-2xl),
        # xl*(-2xh) per coord, then 1*sqh + 1*sql.
        rhs3[b, 0, :np_] = j0h
        rhs3[b, 1, :np_] = j0l
        rhs3[b, 2, :np_] = j0h
        rhs3[b, 3, :np_] = j1h
        rhs3[b, 4, :np_] = j1l
        rhs3[b, 5, :np_] = j1h
        rhs3[b, 6, :np_] = sqh
        rhs3[b, 7, :np_] = sql

        featT[b, 0, :] = placed_f[b]   # exact in fp16 (0/1)
        featT[b, 1, :] = placed_f[b]
        featT[b, 6, :] = 1.0
        featT[b, 7, :] = 1.0
        th_pm[b] = thresh[b].reshape(128, C)

    W = W.astype(np.float32)
    rhsW = np.zeros((8, 384), np.float16)
    rows = [W[0],                         # placed
            W[1] / (2.0 * SCALE),         # S (ACT sign-sum)
            W[1] / SCALE,                 # C (DVE count)
            (wa / (2.0 * SCALE)) * W[1]]  # ones (Sign affine fix)
    for r, v in enumerate(rows):
        h, lo = _split16(v)
        rhsW[2 * r] = h
        rhsW[2 * r + 1] = lo

    return {"xi": xi, "rhs3": rhs3, "thresh": th_pm, "featT": featT,
            "rhsW": rhsW}


_PROGRAM_CACHE = {}


def kernel(action_mask, keepout, probe, locs, W, _trace=False, _tmpdir=None):
    action_mask = np.asarray(action_mask)
    keepout = np.asarray(keepout)
    probe = np.asarray(probe)
    locs = np.asarray(locs, dtype=np.float32)
    W = np.asarray(W, dtype=np.float32)

    B, N = action_mask.shape
    BPC = B // N_CORES

    placed = (~action_mask) & ~(keepout | probe)
    max_placed = int(placed.sum(1).max())
    J = max(1536, ((max_placed + 63) // 64) * 64)
    wa = _wa_for(J)

    key = (N, BPC, J, wa)
    if key not in _PROGRAM_CACHE:
        _PROGRAM_CACHE[key] = build_program(N, BPC, J, wa)
    nc = _PROGRAM_CACHE[key]

    in_maps = []
    for c in range(N_CORES):
        s = slice(c * BPC, (c + 1) * BPC)
        in_maps.append(prep_core_inputs(
            action_mask[s], keepout[s], probe[s], locs[s], W, J, wa))

    res = bass_utils.run_bass_kernel_spmd(
        nc, in_maps, core_ids=list(range(N_CORES)),
        trace=_trace, tmpdir=_tmpdir)

    proj = np.concatenate([res.results[c]["proj"] for c in range(N_CORES)], 0)
    out = (np.ascontiguousarray(proj[:, :, :128]),
           np.ascontiguousarray(proj[:, :, 128:256]),
           np.ascontiguousarray(proj[:, :, 256:384]))
    if _trace:
        return out, res
    return out
